# revision 15
# baseline (speedup 1.0000x reference)
"""CenterLoss kernel for Trainium2, data-parallel across 8 NeuronCores.

Math: the reference masks the full [B, C] squared-distance matrix with
one_hot(labels) and clamps to [1e-12, 1e12] before summing.  The mask keeps
only distmat[i, labels[i]]; every other entry becomes clip(0) = 1e-12, so

    loss = ( sum_i ||x_i - c_{l_i}||^2 + B*(C-1)*1e-12 ) / B

Per core (B/8 = 2048 rows), raw bass pipeline, r-major row layout
(shard row r*128 + p lives at partition p, chunk r; r in [0,16)).

v3 design notes (vs the 39us v1):
  - v1 issued 16 indirect_dma_start's of 128 rows each; each pays ~1.1us
    of SWDGE fixed overhead serialized on the Pool engine, 23us total.
    The HW indirect-DMA ucode reads ONE index per partition per
    instruction (2D offset APs silently mis-gather on HW + OOB-crash), so
    that path caps at 128 rows/instruction.  v3 instead uses
    InstDMAGatherAnt (dma_gather, 'mlp' Q7 library): one instruction
    gathers 1024 rows (~1.4us Pool time).  2048-idx gathers crash the Q7
    ucode on HW (ring reclaim); 1024 verified good -> NG=2 gathers.
  - x and centers ship as fp8 e4m3 (host cast): host-simulated rel err
    7.3e-4 vs the 2e-2 tolerance.  Halves DMA drain vs bf16.
  - diff = x - c on DVE (fp8 in, bf16 out, 2x mode ~1.9 elem/ns/part);
    sum(diff^2) split between ACT Square+accum (dtype-independent
    ~1.2 elem/ns/part) and DVE tensor_tensor_reduce(mult, add) to
    balance the engines: 12 chunks on ACT, 4 on DVE.
"""

import os
import sys
from contextlib import ExitStack

import ml_dtypes
import numpy as np

try:
    import concourse.bass  # noqa: F401
except ImportError:
    sys.path.insert(0, "/opt/trn_rl_repo")

import concourse.bass as bass
import concourse.mybir as mybir
from concourse.bacc import Bacc
from concourse.bass_utils import run_bass_kernel_spmd
from concourse.library_config import mlp as _mlp_lib

B, C, D = 16384, 1000, 512
N_CORES = 8
B_SHARD = B // N_CORES  # 2048
P = 128
NCHUNK = B_SHARD // P  # 16 chunks, chunk r = rows {r*128 + p}
NG = 2  # gather instructions (1024 idxs each; 2048 crashes the Q7 ucode)
GCHUNK = NCHUNK // NG  # 8 chunks per gather
GROWS = B_SHARD // NG  # 1024 rows per gather
NQ = 4  # compute quads (4 chunks each)
QCH = NCHUNK // NQ
CLAMP_MIN = 1e-12

# bisect flags (KV=nottr -> ACT handles all quads; KV=bf16 -> bf16 + nottr)
_KV = os.environ.get("KV", "full")
NOTTR = _KV in ("nottr", "bf16")
if _KV == "bf16":
    FP8 = mybir.dt.bfloat16
    NP_FP8 = ml_dtypes.bfloat16
else:
    FP8 = mybir.dt.float8e4
    NP_FP8 = ml_dtypes.float8_e4m3fn

_NC_CACHE = {}


def build_nc():
    nc = Bacc()
    f32 = mybir.dt.float32
    bf16 = mybir.dt.bfloat16
    x_d = nc.declare_dram_parameter("x", [P, NCHUNK, D], FP8, isOutput=False)
    idx_d = nc.declare_dram_parameter(
        "idx", [P, NCHUNK * 8], mybir.dt.int16, isOutput=False
    )
    cen_d = nc.declare_dram_parameter("centers", [C, D], FP8, isOutput=False)
    out_d = nc.declare_dram_parameter("out", [P, NQ], f32, isOutput=True)

    with ExitStack() as ctx:
        x_sb = ctx.enter_context(nc.sbuf_tensor("x_sb", [P, NCHUNK, D], FP8))
        g_sb = ctx.enter_context(nc.sbuf_tensor("g_sb", [P, NCHUNK, D], FP8))
        diff_sb = ctx.enter_context(nc.sbuf_tensor("diff_sb", [P, NQ, QCH, D], bf16))
        sqa_sb = ctx.enter_context(nc.sbuf_tensor("sqa_sb", [P, QCH, D], bf16))
        sqv_sb = ctx.enter_context(nc.sbuf_tensor("sqv_sb", [P, QCH, D], bf16))
        idx_sb = ctx.enter_context(
            nc.sbuf_tensor("idx_sb", [P, NCHUNK * 8], mybir.dt.int16)
        )
        acc_sb = ctx.enter_context(nc.sbuf_tensor("acc_sb", [P, NQ], f32))

        block = ctx.enter_context(nc.Block())
        ls = ctx.enter_context(nc.semaphore("ls"))
        xs = [ctx.enter_context(nc.semaphore(f"xs{k}")) for k in range(NG)]
        gs = [ctx.enter_context(nc.semaphore(f"gs{k}")) for k in range(NG)]
        vs = ctx.enter_context(nc.semaphore("vs"))
        ss = ctx.enter_context(nc.semaphore("ss"))
        ds = ctx.enter_context(nc.semaphore("ds"))
        os_ = ctx.enter_context(nc.semaphore("os"))

        @block.sync
        def _(sync):
            # idx first: it gates the gather stream (the critical path)
            sync.dma_start(out=idx_sb[:], in_=idx_d[:]).then_inc(ls, 16)
            for k in range(NG):
                sync.dma_start(
                    out=x_sb[:, k * GCHUNK : (k + 1) * GCHUNK, :],
                    in_=x_d[:, k * GCHUNK : (k + 1) * GCHUNK, :],
                ).then_inc(xs[k], 16)

        @block.gpsimd
        def _(gpsimd):
            # dma_gather ucode lives in the 'mlp' Q7 library; load it while
            # the idx DMA is in flight
            gpsimd.load_library(_mlp_lib)
            gpsimd.wait_ge(ls, 16)
            for k in range(NG):
                gpsimd.dma_gather(
                    g_sb[:, k * GCHUNK : (k + 1) * GCHUNK, :],
                    cen_d[:],
                    idx_sb[:, k * (GROWS // 16) : (k + 1) * (GROWS // 16)],
                    GROWS,
                    GROWS,
                    D,
                ).then_inc(gs[k], 16)

        @block.vector
        def _(vector):
            # quads q=0..3; gather half h=q//2 gates quads 2h, 2h+1.
            # DVE: 4 subtracts + quad 1's square-reduce (ttr); ACT: quads
            # 0, 2, 3
            for q in range(NQ):
                vector.wait_ge(xs[q // 2], 16)
                vector.wait_ge(gs[q // 2], 16)
                vector.tensor_tensor(
                    out=diff_sb[:, q, :, :],
                    in0=x_sb[:, q * QCH : (q + 1) * QCH, :],
                    in1=g_sb[:, q * QCH : (q + 1) * QCH, :],
                    op=mybir.AluOpType.subtract,
                ).then_inc(vs, 1)
                if q == 1 and not NOTTR:
                    # same-engine in-order; wait is free but satisfies the
                    # race detector
                    vector.wait_ge(vs, 2)
                    vector.tensor_tensor_reduce(
                        out=sqv_sb[:, :, :],
                        in0=diff_sb[:, 1, :, :],
                        in1=diff_sb[:, 1, :, :],
                        scale=1.0,
                        scalar=0.0,
                        op0=mybir.AluOpType.mult,
                        op1=mybir.AluOpType.add,
                        accum_out=acc_sb[:, 1:2],
                    ).then_inc(ds, 1)

        @block.scalar
        def _(scalar):
            act_quads = (0, 1, 2, 3) if NOTTR else (0, 2, 3)
            for j, q in enumerate(act_quads):
                scalar.wait_ge(vs, q + 1)
                if j:
                    # shared sqa dummy: same-engine WAW, free wait for the
                    # race detector
                    scalar.wait_ge(ss, j)
                scalar.activation(
                    out=sqa_sb[:, :, :],
                    in_=diff_sb[:, q, :, :],
                    func=mybir.ActivationFunctionType.Square,
                    accum_out=acc_sb[:, q : q + 1],
                ).then_inc(ss, 1)
            scalar.wait_ge(ss, len(act_quads))
            if not NOTTR:
                scalar.wait_ge(ds, 1)
            scalar.dma_start(out=out_d[:], in_=acc_sb[:]).then_inc(os_, 16)
            scalar.wait_ge(os_, 16)

    nc.finalize()
    return nc


def _get_nc():
    if "nc" not in _NC_CACHE:
        _NC_CACHE["nc"] = build_nc()
    return _NC_CACHE["nc"]


def _make_idx(lab_shard_i16):
    # gather k consumes idx columns [k*64, (k+1)*64); within a gather,
    # index j' (0..1023) is read from [j' % 16, j' // 16] (16-partition
    # wrap, replicated x8 for the 8 Q7 cores)
    blocks = [
        lab_shard_i16[k * GROWS : (k + 1) * GROWS].reshape(GROWS // 16, 16).T
        for k in range(NG)
    ]
    idx16 = np.hstack(blocks)  # [16, 128]
    return np.ascontiguousarray(np.tile(idx16, (8, 1)))  # [128, 128]


def kernel(x, labels, centers, _trace=False):
    x8 = np.asarray(x, dtype=np.float32).astype(NP_FP8)
    cen8 = np.asarray(centers, dtype=np.float32).astype(NP_FP8)
    labels_i = np.asarray(labels).astype(np.int16)

    in_maps = []
    for i in range(N_CORES):
        xs_ = x8[i * B_SHARD : (i + 1) * B_SHARD]
        # row r*128 + p at [p, r, :]
        xs_ = np.ascontiguousarray(xs_.reshape(NCHUNK, P, D).transpose(1, 0, 2))
        ls_ = labels_i[i * B_SHARD : (i + 1) * B_SHARD]
        in_maps.append(
            {
                "x": xs_,
                "idx": _make_idx(ls_),
                "centers": cen8,
            }
        )

    nc = _get_nc()
    res = run_bass_kernel_spmd(nc, in_maps, list(range(N_CORES)), trace=_trace)
    partials = np.stack([r["out"] for r in res.results])  # [8, 128, NQ]
    total = np.sum(partials.astype(np.float64))
    total += B * (C - 1) * CLAMP_MIN
    loss = np.float32(total / B)
    if _trace:
        return np.asarray(loss), res
    return np.asarray(loss)


# revision 17
# speedup vs baseline: 1.3117x; 1.3117x over previous
"""CenterLoss kernel for Trainium2, data-parallel across 8 NeuronCores.

Math: the reference masks the full [B, C] squared-distance matrix with
one_hot(labels) and clamps to [1e-12, 1e12] before summing.  The mask keeps
only distmat[i, labels[i]]; every other entry becomes clip(0) = 1e-12, so

    loss = ( sum_i ||x_i - c_{l_i}||^2 + B*(C-1)*1e-12 ) / B

Per core (B/8 = 2048 rows), raw bass pipeline, p-major row layout
(shard row 16*p + r lives at partition p, chunk r; r in [0,16)).

The critical path is Q7 descriptor emission for the 2048-row center gather:
~9.1 ns/row on the resident indirect-DMA ucode, serialized on the Pool
engine.  Measured alternatives are all worse:
  - InstDMAGatherAnt (dma_gather): 10.3 ns/idx emission PLUS ~8.6 us of
    one-time 'mlp' Q7 library load inside the measured window; crashes
    outright at num_idxs=2048 (1024 max).
  - 2D offset APs on indirect_dma_start: the HW ucode reads ONE index per
    partition per instruction (sim diverges; OOB-crashes the exec unit).
  - tensor_tensor_reduce: crashes the DVE on this runtime build.
  - fp8: halves DMA drain (not the bottleneck) but drops DVE tensor_tensor
    from 2x to 1x mode -> slower compute, worse accuracy.  bf16 kept.
So: 16 back-to-back indirect_dma_start's (128 rows each, ~1.1-1.2 us),
everything else hidden under that stream:
  - labels load issued from the Pool engine itself (SWDGE) as its first
    instruction -> gather stream starts ~1 us earlier than via sync.
  - dynamic_dma_scratch_size=64KB quadruples the SWDGE descriptor ring to
    reduce Q7 ring-reclaim stalls while x-loads keep the SDMA engines busy.
  - vector/scalar consume at 2-chunk granularity on pair semaphores;
    acc columns 0-5 are stored early to hide the final DMA receipt.
"""

import sys
from contextlib import ExitStack

import ml_dtypes
import numpy as np

try:
    import concourse.bass  # noqa: F401
except ImportError:
    sys.path.insert(0, "/opt/trn_rl_repo")

import concourse.bass as bass
import concourse.mybir as mybir
from concourse.bacc import Bacc
from concourse.bass_utils import run_bass_kernel_spmd

B, C, D = 16384, 1000, 512
N_CORES = 8
B_SHARD = B // N_CORES  # 2048
P = 128
NCHUNK = B_SHARD // P  # 16 chunks, chunk r = rows {16p + r}
NPAIR = NCHUNK // 2  # 8 compute pairs
CLAMP_MIN = 1e-12

_NC_CACHE = {}


def build_nc():
    nc = Bacc(dynamic_dma_scratch_size=2**16)
    f32 = mybir.dt.float32
    bf16 = mybir.dt.bfloat16
    x_d = nc.declare_dram_parameter("x", [B_SHARD, D], bf16, isOutput=False)
    lbl_d = nc.declare_dram_parameter(
        "labels", [P, NCHUNK], mybir.dt.int32, isOutput=False
    )
    cen_d = nc.declare_dram_parameter("centers", [C, D], bf16, isOutput=False)
    out_d = nc.declare_dram_parameter("out", [P, NPAIR], f32, isOutput=True)

    x_r = x_d.rearrange("(p r) d -> p r d", p=P)  # [128, 16, 512]

    with ExitStack() as ctx:
        x_sb = ctx.enter_context(nc.sbuf_tensor("x_sb", [P, NCHUNK, D], bf16))
        g_sb = ctx.enter_context(nc.sbuf_tensor("g_sb", [P, NCHUNK, D], bf16))
        diff_sb = ctx.enter_context(nc.sbuf_tensor("diff_sb", [P, 2, 2, D], bf16))
        sq_sb = ctx.enter_context(nc.sbuf_tensor("sq_sb", [P, 2, D], bf16))
        lbl_sb = ctx.enter_context(
            nc.sbuf_tensor("lbl_sb", [P, NCHUNK], mybir.dt.int32)
        )
        acc_sb = ctx.enter_context(nc.sbuf_tensor("acc_sb", [P, NPAIR], f32))

        block = ctx.enter_context(nc.Block())
        ls = ctx.enter_context(nc.semaphore("ls"))
        xs = [ctx.enter_context(nc.semaphore(f"xs{q}")) for q in range(2)]
        gs = [ctx.enter_context(nc.semaphore(f"gs{k}")) for k in range(NPAIR)]
        vs = ctx.enter_context(nc.semaphore("vs"))
        ss = ctx.enter_context(nc.semaphore("ss"))
        os_ = ctx.enter_context(nc.semaphore("os"))

        @block.sync
        def _(sync):
            for q in range(2):
                sync.dma_start(
                    out=x_sb[:, q * 8 : (q + 1) * 8, :],
                    in_=x_r[:, q * 8 : (q + 1) * 8, :],
                ).then_inc(xs[q], 16)

        @block.gpsimd
        def _(gpsimd):
            # SWDGE-issued labels load: lands ~1us earlier than via sync
            # (Pool is free this early; the gather stream is gated on it)
            gpsimd.dma_start(out=lbl_sb[:], in_=lbl_d[:]).then_inc(ls, 16)
            gpsimd.wait_ge(ls, 16)
            for r in range(NCHUNK):
                gpsimd.indirect_dma_start(
                    out=g_sb[:, r, :],
                    out_offset=None,
                    in_=cen_d[:],
                    in_offset=bass.IndirectOffsetOnAxis(
                        ap=lbl_sb[:, r : r + 1], axis=0
                    ),
                ).then_inc(gs[r // 2], 16)

        @block.vector
        def _(vector):
            for k in range(NPAIR):
                vector.wait_ge(xs[k // 4], 16)
                vector.wait_ge(gs[k], 32)  # both chunks of the pair landed
                if k >= 2:
                    vector.wait_ge(ss, k - 1)  # WAR: scalar done with diff slot
                vector.tensor_tensor(
                    out=diff_sb[:, k % 2, :, :],
                    in0=x_sb[:, 2 * k : 2 * k + 2, :],
                    in1=g_sb[:, 2 * k : 2 * k + 2, :],
                    op=mybir.AluOpType.subtract,
                ).then_inc(vs, 1)

        @block.scalar
        def _(scalar):
            for k in range(NPAIR):
                scalar.wait_ge(vs, k + 1)
                if k:
                    # shared sq dummy: same-engine WAW, free wait for the
                    # race detector
                    scalar.wait_ge(ss, k)
                scalar.activation(
                    out=sq_sb[:, :, :],
                    in_=diff_sb[:, k % 2, :, :],
                    func=mybir.ActivationFunctionType.Square,
                    accum_out=acc_sb[:, k : k + 1],
                ).then_inc(ss, 1)
                if k == NPAIR - 3:
                    # early store of the first 6 columns hides most of the
                    # final DMA's completion receipt behind the last pairs.
                    # ss fires on ACTIVATION_READ_ACCUMULATOR completion, so
                    # this wait orders the store after the accum writes (the
                    # DMA trigger otherwise races the accumulator read-out).
                    scalar.wait_ge(ss, NPAIR - 2)
                    scalar.dma_start(
                        out=out_d[:, : NPAIR - 2], in_=acc_sb[:, : NPAIR - 2]
                    ).then_inc(os_, 16)
            scalar.wait_ge(ss, NPAIR)
            scalar.dma_start(
                out=out_d[:, NPAIR - 2 :], in_=acc_sb[:, NPAIR - 2 :]
            ).then_inc(os_, 16)
            scalar.wait_ge(os_, 32)

    nc.finalize()
    return nc


def _get_nc():
    if "nc" not in _NC_CACHE:
        _NC_CACHE["nc"] = build_nc()
    return _NC_CACHE["nc"]


def kernel(x, labels, centers, _trace=False):
    x = np.asarray(x, dtype=np.float32).astype(ml_dtypes.bfloat16)
    centers = np.asarray(centers, dtype=np.float32).astype(ml_dtypes.bfloat16)
    labels_i = np.asarray(labels).astype(np.int32)

    in_maps = []
    for i in range(N_CORES):
        xs_ = np.ascontiguousarray(x[i * B_SHARD : (i + 1) * B_SHARD])
        ls_ = labels_i[i * B_SHARD : (i + 1) * B_SHARD]
        in_maps.append(
            {
                "x": xs_,
                # row 16p + r at [p, r]
                "labels": np.ascontiguousarray(ls_.reshape(P, NCHUNK)),
                "centers": centers,
            }
        )

    nc = _get_nc()
    res = run_bass_kernel_spmd(nc, in_maps, list(range(N_CORES)), trace=_trace)
    partials = np.stack([r["out"] for r in res.results])  # [8, 128, 8]
    total = np.sum(partials.astype(np.float64))
    total += B * (C - 1) * CLAMP_MIN
    loss = np.float32(total / B)
    if _trace:
        return np.asarray(loss), res
    return np.asarray(loss)


# revision 21
# speedup vs baseline: 1.4989x; 1.1427x over previous
"""CenterLoss kernel for Trainium2, data-parallel across 8 NeuronCores.

Math: the reference masks the full [B, C] squared-distance matrix with
one_hot(labels) and clamps to [1e-12, 1e12] before summing.  The mask keeps
only distmat[i, labels[i]]; every other entry becomes clip(0) = 1e-12, so

    loss = ( sum_i ||x_i - c_{l_i}||^2 + B*(C-1)*1e-12 ) / B

Per core (B/8 = 2048 rows), raw bass pipeline, p-major row layout
(shard row 16*p + r lives at partition p, chunk r; r in [0,16)).

The critical path is Q7 descriptor emission for the 2048-row center gather:
~9.1 ns/row on the resident indirect-DMA ucode, serialized on the Pool
engine.  Measured alternatives are all worse:
  - InstDMAGatherAnt (dma_gather): 10.3 ns/idx emission PLUS ~8.6 us of
    one-time 'mlp' Q7 library load inside the measured window; crashes
    outright at num_idxs=2048 (1024 max).
  - 2D offset APs on indirect_dma_start: the HW ucode reads ONE index per
    partition per instruction (sim diverges; OOB-crashes the exec unit).
  - tensor_tensor_reduce: crashes the DVE on this runtime build.
  - fp8: halves DMA drain (not the bottleneck) but drops DVE tensor_tensor
    from 2x to 1x mode -> slower compute, worse accuracy.  bf16 kept.
So: 16 back-to-back indirect_dma_start's (128 rows each, ~1.1-1.2 us),
everything else hidden under that stream:
  - labels load issued from the Pool engine itself (SWDGE) as its first
    instruction -> gather stream starts ~1 us earlier than via sync.
  - dynamic_dma_scratch_size=64KB quadruples the SWDGE descriptor ring to
    reduce Q7 ring-reclaim stalls while x-loads keep the SDMA engines busy.
  - vector/scalar consume at 2-chunk granularity on pair semaphores;
    acc columns 0-5 are stored early to hide the final DMA receipt.
"""

import sys
from contextlib import ExitStack

import ml_dtypes
import numpy as np

try:
    import concourse.bass  # noqa: F401
except ImportError:
    sys.path.insert(0, "/opt/trn_rl_repo")

import concourse.bass as bass
import concourse.mybir as mybir
from concourse.bacc import Bacc
from concourse.bass_utils import run_bass_kernel_spmd

B, C, D = 16384, 1000, 512
N_CORES = 8
B_SHARD = B // N_CORES  # 2048
P = 128
NCHUNK = B_SHARD // P  # 16 chunks, chunk r = rows {16p + r}
NPAIR = NCHUNK // 2  # 8 compute pairs
NFULL = NPAIR - 1  # pairs 0..6 full-size; chunks 14/15 run singly (tail)
NCOL = NFULL + 2  # 9 accumulator columns
CLAMP_MIN = 1e-12

import os

SEMHALF = os.environ.get("SEMHALF", "0") == "1"

_NC_CACHE = {}


def build_nc():
    nc = Bacc(dynamic_dma_scratch_size=2**16)
    f32 = mybir.dt.float32
    bf16 = mybir.dt.bfloat16
    x_d = nc.declare_dram_parameter("x", [B_SHARD, D], bf16, isOutput=False)
    lbl_d = nc.declare_dram_parameter(
        "labels", [P, NCHUNK], mybir.dt.int32, isOutput=False
    )
    cen_d = nc.declare_dram_parameter("centers", [C, D], bf16, isOutput=False)
    out_d = nc.declare_dram_parameter("out", [P, NCOL], f32, isOutput=True)

    x_r = x_d.rearrange("(p r) d -> p r d", p=P)  # [128, 16, 512]

    with ExitStack() as ctx:
        x_sb = ctx.enter_context(nc.sbuf_tensor("x_sb", [P, NCHUNK, D], bf16))
        g_sb = ctx.enter_context(nc.sbuf_tensor("g_sb", [P, NCHUNK, D], bf16))
        diff_sb = ctx.enter_context(nc.sbuf_tensor("diff_sb", [P, 3, 2, D], bf16))
        sq_sb = ctx.enter_context(nc.sbuf_tensor("sq_sb", [P, 2, D], bf16))
        lbl_sb = ctx.enter_context(
            nc.sbuf_tensor("lbl_sb", [P, NCHUNK], mybir.dt.int32)
        )
        acc_sb = ctx.enter_context(nc.sbuf_tensor("acc_sb", [P, NCOL], f32))

        block = ctx.enter_context(nc.Block())
        ls = ctx.enter_context(nc.semaphore("ls"))
        xs = [ctx.enter_context(nc.semaphore(f"xs{q}")) for q in range(2)]
        gs = [ctx.enter_context(nc.semaphore(f"gs{k}")) for k in range(NFULL)]
        gt = [ctx.enter_context(nc.semaphore(f"gt{j}")) for j in range(2)]
        vs = ctx.enter_context(nc.semaphore("vs"))
        ss = ctx.enter_context(nc.semaphore("ss"))
        os_ = ctx.enter_context(nc.semaphore("os"))

        @block.sync
        def _(sync):
            # labels first: they drain ahead of x on the same HWDGE queue
            # (FIFO per queue), gating the gather stream as early as possible
            sync.dma_start(out=lbl_sb[:], in_=lbl_d[:]).then_inc(ls, 16)
            for q in range(2):
                sync.dma_start(
                    out=x_sb[:, q * 8 : (q + 1) * 8, :],
                    in_=x_r[:, q * 8 : (q + 1) * 8, :],
                ).then_inc(xs[q], 16)

        @block.gpsimd
        def _(gpsimd):
            gpsimd.wait_ge(ls, 16)
            for r in range(NCHUNK):
                dma = gpsimd.indirect_dma_start(
                    out=g_sb[:, r, :],
                    out_offset=None,
                    in_=cen_d[:],
                    in_offset=bass.IndirectOffsetOnAxis(
                        ap=lbl_sb[:, r : r + 1], axis=0
                    ),
                )
                if r >= 14:
                    # last two chunks signal individually: shortens the
                    # tail (compute consumes them singly)
                    dma.then_inc(gt[r - 14], 16)
                elif SEMHALF:
                    # sem only on the odd chunk of each pair: qPoolDynamic
                    # is FIFO per engine ring, so chunk 2k's descriptors
                    # complete before chunk 2k+1's final descriptor on
                    # every engine.  (Invisible to the sim race detector —
                    # HW-only experiment.)
                    if r % 2 == 1:
                        dma.then_inc(gs[r // 2], 16)
                else:
                    dma.then_inc(gs[r // 2], 16)

        @block.vector
        def _(vector):
            for k in range(NFULL):
                vector.wait_ge(xs[k // 4], 16)
                # both chunks of the pair landed
                vector.wait_ge(gs[k], 16 if SEMHALF else 32)
                if k >= 2:
                    vector.wait_ge(ss, k - 1)  # WAR: scalar done with diff slot
                vector.tensor_tensor(
                    out=diff_sb[:, k % 2, :, :],
                    in0=x_sb[:, 2 * k : 2 * k + 2, :],
                    in1=g_sb[:, 2 * k : 2 * k + 2, :],
                    op=mybir.AluOpType.subtract,
                ).then_inc(vs, 1)
            for j in range(2):  # chunks 14, 15 singly into slot 2
                vector.wait_ge(xs[1], 16)
                vector.wait_ge(gt[j], 16)
                vector.tensor_tensor(
                    out=diff_sb[:, 2, j : j + 1, :],
                    in0=x_sb[:, 14 + j : 15 + j, :],
                    in1=g_sb[:, 14 + j : 15 + j, :],
                    op=mybir.AluOpType.subtract,
                ).then_inc(vs, 1)

        @block.scalar
        def _(scalar):
            for k in range(NFULL):
                scalar.wait_ge(vs, k + 1)
                if k:
                    # shared sq dummy: same-engine WAW, free wait for the
                    # race detector
                    scalar.wait_ge(ss, k)
                scalar.activation(
                    out=sq_sb[:, :, :],
                    in_=diff_sb[:, k % 2, :, :],
                    func=mybir.ActivationFunctionType.Square,
                    accum_out=acc_sb[:, k : k + 1],
                ).then_inc(ss, 1)
                if k == NFULL - 2:
                    # early store of the first 5 columns hides most of the
                    # final DMA's completion receipt behind the last pairs.
                    # ss fires on ACTIVATION_READ_ACCUMULATOR completion, so
                    # this wait orders the store after the accum writes (the
                    # DMA trigger otherwise races the accumulator read-out).
                    scalar.wait_ge(ss, NFULL - 1)
                    scalar.dma_start(
                        out=out_d[:, : NFULL - 1], in_=acc_sb[:, : NFULL - 1]
                    ).then_inc(os_, 16)
            for j in range(2):  # chunks 14, 15 singly (short tail ops)
                scalar.wait_ge(vs, NFULL + j + 1)
                scalar.wait_ge(ss, NFULL + j)
                scalar.activation(
                    out=sq_sb[:, j : j + 1, :],
                    in_=diff_sb[:, 2, j : j + 1, :],
                    func=mybir.ActivationFunctionType.Square,
                    accum_out=acc_sb[:, NFULL + j : NFULL + j + 1],
                ).then_inc(ss, 1)
            scalar.wait_ge(ss, NFULL + 2)
            scalar.dma_start(
                out=out_d[:, NFULL - 1 :], in_=acc_sb[:, NFULL - 1 :]
            ).then_inc(os_, 16)
            scalar.wait_ge(os_, 32)

    nc.finalize()
    return nc


def _get_nc():
    if "nc" not in _NC_CACHE:
        _NC_CACHE["nc"] = build_nc()
    return _NC_CACHE["nc"]


def kernel(x, labels, centers, _trace=False):
    x = np.asarray(x, dtype=np.float32).astype(ml_dtypes.bfloat16)
    centers = np.asarray(centers, dtype=np.float32).astype(ml_dtypes.bfloat16)
    labels_i = np.asarray(labels).astype(np.int32)

    in_maps = []
    for i in range(N_CORES):
        xs_ = np.ascontiguousarray(x[i * B_SHARD : (i + 1) * B_SHARD])
        ls_ = labels_i[i * B_SHARD : (i + 1) * B_SHARD]
        in_maps.append(
            {
                "x": xs_,
                # row 16p + r at [p, r]
                "labels": np.ascontiguousarray(ls_.reshape(P, NCHUNK)),
                "centers": centers,
            }
        )

    nc = _get_nc()
    res = run_bass_kernel_spmd(nc, in_maps, list(range(N_CORES)), trace=_trace)
    partials = np.stack([r["out"] for r in res.results])  # [8, 128, NCOL]
    total = np.sum(partials.astype(np.float64))
    total += B * (C - 1) * CLAMP_MIN
    loss = np.float32(total / B)
    if _trace:
        return np.asarray(loss), res
    return np.asarray(loss)
